# revision 8
# baseline (speedup 1.0000x reference)
import numpy as np

# nn_NearestNeighbours: batch [8,512,512] f32, emb [50000,512] f32,
# output argmin indices [8,512] int32. Vocab-sharded across 8 cores.
# Screen: fp8e4m3 DoubleRow GEMM -> f16 scores (contiguous evict on ACT)
# -> DVE f16 pairwise-max tree 6272->3136->1568->784->392 (2x mode)
# -> DVE top-8 + max_index over 392 group maxima -> host expands each
# survivor group (16 members) and rescores exactly in f32 cosine.
B, S, E, V = 8, 512, 512, 50000
R = B * S              # 4096 token rows
NC = 8                 # cores
VS = V // NC           # 6250 vocab rows per core
VSP = 6272             # padded: 4*1536 + 128, = 16*392
L1, L2, L3, L4 = 3136, 1568, 784, 392
KT = E // 128          # 4 k-subtiles
MT = R // 128          # 32 m-tiles
GW = [1536, 1536, 1536, 1536]  # psA group widths; +128 tail in psB
TOPK = 20              # survivors rescored exactly on host (of 64)

_CACHE = {}


def _build():
    import concourse.bacc as bacc
    import concourse.mybir as mybir
    from concourse.tile import TileContext

    dtf = mybir.dt.float32
    dt8 = mybir.dt.float8e4
    dth = mybir.dt.float16
    dtu16 = mybir.dt.uint16
    DR = mybir.MatmulPerfMode.DoubleRow

    nc = bacc.Bacc("TRN2", target_bir_lowering=False, debug=False)
    bT_ap = nc.dram_tensor("bT", [E, R], dt8, kind="ExternalInput").ap()
    embT_ap = nc.dram_tensor("embT", [E, VSP], dt8, kind="ExternalInput").ap()
    vals_ap = nc.dram_tensor("vals", [R, 8], dth, kind="ExternalOutput").ap()
    idxs_ap = nc.dram_tensor("idxs", [R, 8], dtu16, kind="ExternalOutput").ap()

    with TileContext(nc) as tc:
        with tc.sbuf_pool(name="emb", bufs=1) as embp, \
             tc.sbuf_pool(name="bt", bufs=2) as btp, \
             tc.sbuf_pool(name="sc", bufs=2) as scp, \
             tc.sbuf_pool(name="tr", bufs=2) as trp, \
             tc.sbuf_pool(name="out", bufs=2) as outp, \
             tc.psum_pool(name="psA", bufs=2) as psA, \
             tc.psum_pool(name="psB", bufs=2) as psB:
            gt = btp.tile([128, KT, 512], dt8)
            for k in range(KT):
                nc.scalar.dma_start(gt[:, k:k + 1, :],
                                    bT_ap[128 * k:128 * (k + 1), 0:512])
            emb8 = embp.tile([128, KT, VSP], dt8, name="emb8")
            off = 0
            for w in [1024] * 6 + [128]:
                eng = nc.scalar if off >= 5120 else nc.sync
                for k in range(KT):
                    eng.dma_start(
                        emb8[:, k:k + 1, off:off + w],
                        embT_ap[128 * k:128 * (k + 1), off:off + w],
                    )
                off += w

            for g in range(MT // 4):
                cur = gt
                if g + 1 < MT // 4:
                    gt = btp.tile([128, KT, 512], dt8)
                    for k in range(KT):
                        nc.sync.dma_start(
                            gt[:, k:k + 1, :],
                            bT_ap[128 * k:128 * (k + 1), 512 * (g + 1):512 * (g + 2)],
                        )
                for mm in range(4):
                    m = g * 4 + mm
                    sc = scp.tile([128, VSP], dth)
                    off = 0
                    for gi, w in enumerate(GW):
                        pt = psA.tile([128, w], dtf)
                        for p in range(2):
                            for c0 in range(0, w, 512):
                                nc.tensor.matmul(
                                    pt[:, c0:c0 + 512],
                                    cur[:, 2 * p:2 * p + 2, 128 * mm:128 * mm + 128],
                                    emb8[:, 2 * p:2 * p + 2,
                                         off + c0:off + c0 + 512],
                                    start=(p == 0),
                                    stop=(p == 1),
                                    perf_mode=DR,
                                )
                        if gi == len(GW) - 1:
                            # split the last group's eviction ACT/DVE to
                            # balance the two engines
                            nc.scalar.copy(sc[:, off:off + 1024], pt[:, 0:1024])
                            nc.vector.tensor_copy(sc[:, off + 1024:off + w],
                                                  pt[:, 1024:w])
                        else:
                            nc.scalar.copy(sc[:, off:off + w], pt[:])
                        off += w
                    pt = psB.tile([128, 128], dtf)
                    for p in range(2):
                        nc.tensor.matmul(
                            pt[:],
                            cur[:, 2 * p:2 * p + 2, 128 * mm:128 * mm + 128],
                            emb8[:, 2 * p:2 * p + 2, off:off + 128],
                            start=(p == 0),
                            stop=(p == 1),
                            perf_mode=DR,
                        )
                    nc.vector.tensor_copy(sc[:, off:off + 128], pt[:])
                    t1 = trp.tile([128, L1], dth)
                    nc.vector.tensor_max(t1[:], sc[:, 0:L1], sc[:, L1:VSP])
                    t2 = trp.tile([128, L2], dth)
                    nc.vector.tensor_max(t2[:], t1[:, 0:L2], t1[:, L2:L1])
                    t3 = trp.tile([128, L3], dth)
                    nc.vector.tensor_max(t3[:], t2[:, 0:L3], t2[:, L3:L2])
                    t4 = trp.tile([128, L4], dth)
                    nc.vector.tensor_max(t4[:], t3[:, 0:L4], t3[:, L4:L3])
                    vt = outp.tile([128, 8], dth)
                    it = outp.tile([128, 8], dtu16)
                    nc.vector.max(vt[:], t4[:])
                    nc.vector.max_index(it[:], vt[:], t4[:])
                    nc.sync.dma_start(vals_ap[128 * m:128 * (m + 1), :], vt[:])
                    nc.sync.dma_start(idxs_ap[128 * m:128 * (m + 1), :], it[:])
    nc.compile()
    return nc


def _run(batch: np.ndarray, emb: np.ndarray, trace: bool = False, **kw):
    import ml_dtypes
    from concourse import bass_utils

    if "nc" not in _CACHE:
        _CACHE["nc"] = _build()
    nc = _CACHE["nc"]
    f8 = ml_dtypes.float8_e4m3

    b = np.ascontiguousarray(batch.reshape(R, E).astype(np.float32))
    bT8 = np.ascontiguousarray(b.T).astype(f8)
    embT8 = emb.T.astype(f8)
    in_maps = []
    for c in range(NC):
        shardT = np.zeros((E, VSP), f8)
        shardT[:, :VS] = embT8[:, c * VS:(c + 1) * VS]
        in_maps.append({"bT": bT8, "embT": shardT})

    res = bass_utils.run_bass_kernel_spmd(
        nc, in_maps, core_ids=list(range(NC)), trace=trace, **kw
    )

    # survivors: 8 group-maxima (value f16, group id p in [0,392)) per core
    svals = np.empty((R, NC * 8), np.float32)
    spos = np.empty((R, NC * 8), np.int64)
    for c in range(NC):
        svals[:, c * 8:(c + 1) * 8] = res.results[c]["vals"].astype(np.float32)
        p = res.results[c]["idxs"].astype(np.int64)
        spos[:, c * 8:(c + 1) * 8] = np.minimum(p, L4 - 1) + c * VSP

    # pick top-K survivors per row, expand each group to its 16 members
    top = np.argpartition(-svals, TOPK, axis=1)[:, :TOPK]          # [R,K]
    rows = np.arange(R)[:, None]
    gpos = spos[rows, top]                                         # [R,K]
    cand = gpos[:, :, None] + L4 * np.arange(16)[None, None, :]    # [R,K,16]
    cand = cand.reshape(R, -1)                                     # padded ids
    core = cand // VSP
    loc = cand - core * VSP
    valid = loc < VS
    gid = np.where(valid, core * VS + np.minimum(loc, VS - 1), 0)

    # exact rescore in f32 on normalized embeddings
    en = emb / np.sqrt((emb * emb).sum(axis=1, keepdims=True))
    bn = b / np.sqrt((b * b).sum(axis=1, keepdims=True))
    best = np.empty(R, np.int64)
    CH = 256
    NCAND = gid.shape[1]
    for r0 in range(0, R, CH):
        r1 = min(r0 + CH, R)
        g = gid[r0:r1]
        ce = en[g]                                   # [ch,NCAND,512]
        s = np.matmul(ce, bn[r0:r1, :, None])[:, :, 0]
        s[~valid[r0:r1]] = -np.inf
        am = np.argmax(s, axis=1)
        best[r0:r1] = g[np.arange(r1 - r0), am]

    return best.astype(np.int32).reshape(B, S), res


def kernel(batch: np.ndarray, emb: np.ndarray) -> np.ndarray:
    out, _ = _run(batch, emb, trace=False)
    return out


# revision 12
# speedup vs baseline: 1.0162x; 1.0162x over previous
import numpy as np

# nn_NearestNeighbours: batch [8,512,512] f32, emb [50000,512] f32,
# output argmin indices [8,512] int32. Vocab-sharded across 8 cores.
# Screen: fp8e4m3 DoubleRow GEMM -> f16 scores (contiguous evict,
# ACT does groups 0-2 + the 128 tail, DVE does group 3) -> DVE f16
# pairwise-max tree 6272->3136->1568->784->392 (2x mode) -> DMA the
# full 392-wide group-max array out; host picks global top-K groups,
# expands each to its 16 members and rescores exactly in f32 cosine.
B, S, E, V = 8, 512, 512, 50000
R = B * S              # 4096 token rows
NC = 8                 # cores
VS = V // NC           # 6250 vocab rows per core
VSP = 6272             # padded: 4*1536 + 128, = 16*392
L1, L2, L3, L4 = 3136, 1568, 784, 392
KT = E // 128          # 4 k-subtiles
MT = R // 128          # 32 m-tiles
GW = [1536, 1536, 1536, 1536]  # psA group widths; +128 tail in psB
TOPK = 20              # survivors rescored exactly on host (of 64)

_CACHE = {}


def _build():
    import concourse.bacc as bacc
    import concourse.mybir as mybir
    from concourse.tile import TileContext

    dtf = mybir.dt.float32
    dt8 = mybir.dt.float8e4
    dth = mybir.dt.float16
    dtu16 = mybir.dt.uint16
    DR = mybir.MatmulPerfMode.DoubleRow

    nc = bacc.Bacc("TRN2", target_bir_lowering=False, debug=False)
    bT_ap = nc.dram_tensor("bT", [E, R], dt8, kind="ExternalInput").ap()
    embT_ap = nc.dram_tensor("embT", [E, VSP], dt8, kind="ExternalInput").ap()
    gm_ap = nc.dram_tensor("gm", [R, L4], dth, kind="ExternalOutput").ap()

    with TileContext(nc) as tc:
        with tc.sbuf_pool(name="emb", bufs=1) as embp, \
             tc.sbuf_pool(name="bt", bufs=2) as btp, \
             tc.sbuf_pool(name="sc", bufs=2) as scp, \
             tc.sbuf_pool(name="tr", bufs=2) as trp, \
             tc.sbuf_pool(name="out", bufs=2) as outp, \
             tc.psum_pool(name="psA", bufs=2) as psA, \
             tc.psum_pool(name="psB", bufs=2) as psB:
            gt = btp.tile([128, KT, 512], dt8)
            for k in range(KT):
                nc.scalar.dma_start(gt[:, k:k + 1, :],
                                    bT_ap[128 * k:128 * (k + 1), 0:512])
            emb8 = embp.tile([128, KT, VSP], dt8, name="emb8")
            off = 0
            for w in [1024] * 6 + [128]:
                eng = nc.scalar if off >= 5120 else nc.sync
                for k in range(KT):
                    eng.dma_start(
                        emb8[:, k:k + 1, off:off + w],
                        embT_ap[128 * k:128 * (k + 1), off:off + w],
                    )
                off += w

            for g in range(MT // 4):
                cur = gt
                if g + 1 < MT // 4:
                    gt = btp.tile([128, KT, 512], dt8)
                    for k in range(KT):
                        nc.sync.dma_start(
                            gt[:, k:k + 1, :],
                            bT_ap[128 * k:128 * (k + 1), 512 * (g + 1):512 * (g + 2)],
                        )
                for mm in range(4):
                    m = g * 4 + mm
                    sc = scp.tile([128, VSP], dth)
                    # tail MMs first: fillers that need no psA slot
                    ptb = psB.tile([128, 128], dtf)
                    for p in range(2):
                        nc.tensor.matmul(
                            ptb[:],
                            cur[:, 2 * p:2 * p + 2, 128 * mm:128 * mm + 128],
                            emb8[:, 2 * p:2 * p + 2, 6144:6272],
                            start=(p == 0),
                            stop=(p == 1),
                            perf_mode=DR,
                        )
                    nc.scalar.copy(sc[:, 6144:6272], ptb[:])
                    off = 0
                    for gi, w in enumerate(GW):
                        pt = psA.tile([128, w], dtf)
                        for p in range(2):
                            for c0 in range(0, w, 512):
                                nc.tensor.matmul(
                                    pt[:, c0:c0 + 512],
                                    cur[:, 2 * p:2 * p + 2, 128 * mm:128 * mm + 128],
                                    emb8[:, 2 * p:2 * p + 2,
                                         off + c0:off + c0 + 512],
                                    start=(p == 0),
                                    stop=(p == 1),
                                    perf_mode=DR,
                                )
                        if gi == len(GW) - 1:
                            nc.vector.tensor_copy(sc[:, off:off + w], pt[:])
                        else:
                            nc.scalar.copy(sc[:, off:off + w], pt[:])
                        off += w
                    t1 = trp.tile([128, L1], dth)
                    nc.vector.tensor_max(t1[:], sc[:, 0:L1], sc[:, L1:VSP])
                    t2 = trp.tile([128, L2], dth)
                    nc.vector.tensor_max(t2[:], t1[:, 0:L2], t1[:, L2:L1])
                    t3 = trp.tile([128, L3], dth)
                    nc.vector.tensor_max(t3[:], t2[:, 0:L3], t2[:, L3:L2])
                    t4 = outp.tile([128, L4], dth)
                    nc.vector.tensor_max(t4[:], t3[:, 0:L4], t3[:, L4:L3])
                    nc.sync.dma_start(gm_ap[128 * m:128 * (m + 1), :], t4[:])
    nc.compile()
    return nc


def _run(batch: np.ndarray, emb: np.ndarray, trace: bool = False, **kw):
    import ml_dtypes
    from concourse import bass_utils

    if "nc" not in _CACHE:
        _CACHE["nc"] = _build()
    nc = _CACHE["nc"]
    f8 = ml_dtypes.float8_e4m3

    b = np.ascontiguousarray(batch.reshape(R, E).astype(np.float32))
    bT8 = np.ascontiguousarray(b.T).astype(f8)
    embT8 = emb.T.astype(f8)
    in_maps = []
    for c in range(NC):
        shardT = np.zeros((E, VSP), f8)
        shardT[:, :VS] = embT8[:, c * VS:(c + 1) * VS]
        in_maps.append({"bT": bT8, "embT": shardT})

    res = bass_utils.run_bass_kernel_spmd(
        nc, in_maps, core_ids=list(range(NC)), trace=trace, **kw
    )

    # group maxima: [R, NC*392] f16; group (c, p) covers ids p+392k, k<16
    gm = np.concatenate([res.results[c]["gm"] for c in range(NC)], axis=1)
    gm = gm.astype(np.float32)                                     # [R, 3136]

    # pick global top-K groups per row, expand each to its 16 members
    top = np.argpartition(-gm, TOPK, axis=1)[:, :TOPK]             # [R,K]
    core = top // L4
    gpos = (top - core * L4) + core * VSP                          # [R,K]
    cand = gpos[:, :, None] + L4 * np.arange(16)[None, None, :]    # [R,K,16]
    cand = cand.reshape(R, -1)                                     # padded ids
    core = cand // VSP
    loc = cand - core * VSP
    valid = loc < VS
    gid = np.where(valid, core * VS + np.minimum(loc, VS - 1), 0)

    # exact rescore in f32 on normalized embeddings
    en = emb / np.sqrt((emb * emb).sum(axis=1, keepdims=True))
    bn = b / np.sqrt((b * b).sum(axis=1, keepdims=True))
    best = np.empty(R, np.int64)
    CH = 256
    NCAND = gid.shape[1]
    for r0 in range(0, R, CH):
        r1 = min(r0 + CH, R)
        g = gid[r0:r1]
        ce = en[g]                                   # [ch,NCAND,512]
        s = np.matmul(ce, bn[r0:r1, :, None])[:, :, 0]
        s[~valid[r0:r1]] = -np.inf
        am = np.argmax(s, axis=1)
        best[r0:r1] = g[np.arange(r1 - r0), am]

    return best.astype(np.int32).reshape(B, S), res


def kernel(batch: np.ndarray, emb: np.ndarray) -> np.ndarray:
    out, _ = _run(batch, emb, trace=False)
    return out
